# revision 26
# baseline (speedup 1.0000x reference)
"""ChannelAttention Trainium2 Bass kernel.

Full (unsharded) inputs -> full output. Data-parallel over batch B=8 across
the 8 NeuronCores (one batch element per core, SPMD program, no collectives).

Per-core math (N=4096 tokens, C=512 channels):
    qkv = x @ Wqkv + bqkv ; q,k,v = relu(split(qkv))
    scores = (q^T k) / sqrt(C)           # [C, C] contraction over tokens
    attn = softmax(scores, -1) * adj
    y = v @ attn ; out = y @ Wo + bo

v2 design (PE-bound, ~768 N=512 matmuls):
  - all matmuls in bf16 (PSUM accumulate f32; rel err ~2e-3 vs 2e-2 gate)
  - x^T built with the DMA crossbar transpose (dma_start_transpose, bf16)
    instead of PE transposes: frees ~30us of PE and ~27us of DVE
  - q/k bias folded via DVE add into PSUM + Act relu (kills 64 bias matmuls)
  - scores matmuls run one token-tile behind q/k to hide Act/sem latency
  - v-projection deferred into pass 2 (overlaps the softmax stall) and
    interleaved two slabs ahead of y/out to hide relu/copy latency
"""

import sys

sys.path.insert(0, "/opt/trn_rl_repo")

from contextlib import ExitStack

import numpy as np

import concourse.bass as bass
import concourse.mybir as mybir
import concourse.tile as tile
from concourse import bacc
from concourse.bass import ds, ts
from concourse.bass_utils import run_bass_kernel_spmd

# Problem shape (hardcoded per contract).
B, N, C = 8, 4096, 512
P = 128
CC = C // P            # channel chunks (4)
NT = N // P            # token tiles (32)
NS = 8                 # pass-2 slabs
TPS = NT // NS         # token tiles per slab (4)
SLAB = TPS * P         # tokens per slab (512)

F32 = mybir.dt.float32
BF16 = mybir.dt.bfloat16
ADD = mybir.AluOpType.add
MULT = mybir.AluOpType.mult
RELU = mybir.ActivationFunctionType.Relu

_CACHE = {}


def build(reps: int = 1, probe: str | None = None, pair_qk: bool = True,
          xbar_act: bool = False, pool_bias: bool = False,
          act_store: bool = True):
    # pool_bias=True (Pool-engine bias add via gpsimd scalar_tensor_tensor)
    # fails walrus codegen on this toolchain; kept for reference, off.
    # act_store issues the out-store DMAs from the Act queue to offload SP.
    nc = bacc.Bacc("TRN2", target_bir_lowering=False, debug=False, num_devices=8)

    x = nc.dram_tensor("x", [N, C], F32, kind="ExternalInput").ap()
    adj = nc.dram_tensor("adj", [C, C], F32, kind="ExternalInput").ap()
    wqkv = nc.dram_tensor("Wqkv", [C, 3 * C], F32, kind="ExternalInput").ap()
    bqkv = nc.dram_tensor("bqkv", [3 * C], F32, kind="ExternalInput").ap()
    wo = nc.dram_tensor("Wo", [C, C], F32, kind="ExternalInput").ap()
    bo = nc.dram_tensor("bo", [C], F32, kind="ExternalInput").ap()
    out = nc.dram_tensor("out", [N, C], F32, kind="ExternalOutput").ap()

    s = 1.0 / float(np.sqrt(C))

    with tile.TileContext(nc) as tc, ExitStack() as ctx:
        const = ctx.enter_context(tc.tile_pool(name="const", bufs=1))
        xprime = ctx.enter_context(tc.tile_pool(name="xprime", bufs=6))
        xbp = ctx.enter_context(tc.tile_pool(name="xb", bufs=6))

        xt_all = const.tile([P, CC, N], BF16)    # x^T, channel-major (32KB/part)
        attn_sb = const.tile([P, CC, C], BF16)   # gated softmax rows
        attn_T = const.tile([P, CC, C], BF16)    # attn^T (d-major) via XBAR
        aw_sb = const.tile([P, CC, C], BF16)     # attn @ Wo, channel-major

        def prep_xt(t, x_t):
            x_b = xbp.tile([P, C], BF16, tag="xb", name=f"xb_{t}")
            nc.vector.tensor_copy(x_b[:], x_t[:])
            eng = nc.scalar if xbar_act else nc.sync
            eng.dma_start_transpose(xt_all[:, :, ts(t, P)], x_b[:])

        # ---- constants; emission order = queue order: small consts and
        # the x-pipeline prologue go first so PE/SP unblock early, big
        # weight DMAs after ------------------------------------------------
        with tc.tile_pool(name="stage", bufs=1) as stage:
            brow_f = stage.tile([1, 2 * C], F32, tag="stage_b")
            nc.sync.dma_start(brow_f[:], bqkv[None, 0 : 2 * C])
            brow_r = stage.tile([1, 2 * C], BF16, tag="stage_br")
            nc.vector.tensor_copy(brow_r[:], brow_f[:])

            borow_f = stage.tile([1, C], F32, tag="stage_bo")
            nc.sync.dma_start(borow_f[:], bo[None, :])
            borow_r = stage.tile([1, C], BF16, tag="stage_bor")
            nc.vector.tensor_copy(borow_r[:], borow_f[:])

            ones_f = stage.tile([1, P], F32, tag="stage_ones")
            nc.gpsimd.memset(ones_f[:], 1.0)
            ones_r = stage.tile([1, P], BF16, tag="stage_onesr")
            nc.vector.tensor_copy(ones_r[:], ones_f[:])

            # broadcast biases to [P, *] once (read along free dim later)
            bias_qk = const.tile([P, 2 * C], F32)
            bo_bc = const.tile([P, C], F32)
            with tc.tile_pool(name="bc_ps", bufs=1, space="PSUM") as bc_pool:
                bq_ps = bc_pool.tile([P, C], F32, name="bq_ps", tag="b0")
                nc.tensor.matmul(bq_ps[:], ones_r[:], brow_r[:, 0:C], start=True, stop=True)
                nc.vector.tensor_copy(bias_qk[:, 0:C], bq_ps[:])
                bk_ps = bc_pool.tile([P, C], F32, name="bk_ps", tag="b1")
                nc.tensor.matmul(bk_ps[:], ones_r[:], brow_r[:, C : 2 * C], start=True, stop=True)
                nc.vector.tensor_copy(bias_qk[:, C : 2 * C], bk_ps[:])
                bo_ps = bc_pool.tile([P, C], F32, name="bo_ps", tag="b2")
                nc.tensor.matmul(bo_ps[:], ones_r[:], borow_r[:], start=True, stop=True)
                nc.vector.tensor_copy(bo_bc[:], bo_ps[:])

            # x-pipeline prologue: tiles 0-5 loaded and 0-3 transposed once,
            # outside the rep loop (software-pipeline prologue; x is
            # loop-invariant so iterations 2+ reuse these slices)
            xpr = {}
            for t in range(6):
                xp = xprime.tile([P, C], F32, tag="xp", name=f"xp_{t}")
                nc.sync.dma_start(xp[:], x[ts(t, P), :])
                xpr[t] = xp
            for t in range(4):
                prep_xt(t, xpr[t])

            # big weights after the prologue, issued from the Act queue so
            # they stream in parallel with the x-prologue DMAs on SP
            wqkv_f = stage.tile([P, CC, 3 * C], F32, tag="stage_wqkv")
            wqkv_r = const.tile([P, CC, 3 * C], BF16)
            for o in range(CC):
                nc.scalar.dma_start(
                    wqkv_f[:, o, :], wqkv.rearrange("(o p) d -> p o d", p=P)[:, o, :]
                )
                nc.vector.tensor_copy(wqkv_r[:, o, :], wqkv_f[:, o, :])

            wo_f = stage.tile([P, CC, C], F32, tag="stage_wo")
            nc.scalar.dma_start(wo_f[:], wo.rearrange("(o p) d -> p o d", p=P))
            wo_r = const.tile([P, CC, C], BF16)
            nc.vector.tensor_copy(wo_r[:], wo_f[:])

        # v-bias, per-partition layout [p, chunk]
        bv = const.tile([P, CC], F32)
        nc.sync.dma_start(bv[:], bqkv[2 * C :].rearrange("(o p) -> p o", p=P))

        adj_sb = const.tile([P, CC, C], F32)
        nc.sync.dma_start(adj_sb[:], adj.rearrange("(o p) d -> p o d", p=P))

        if probe == "noxbar":
            # stage x^T once, outside the rep loop (timing probe: removes
            # per-iteration x DMA + convert + XBAR; outputs stay correct)
            with tc.tile_pool(name="xstage", bufs=3) as xst:
                for t in range(NT):
                    xs = xst.tile([P, C], F32, tag="xs", name=f"xs_{t}")
                    nc.sync.dma_start(xs[:], x[ts(t, P), :])
                    xb = xst.tile([P, C], BF16, tag="xsb", name=f"xsb_{t}")
                    nc.vector.tensor_copy(xb[:], xs[:])
                    nc.sync.dma_start_transpose(xt_all[:, :, ts(t, P)], xb[:])

        # ---- per-iteration body ---------------------------------------
        scores_pool = ctx.enter_context(
            tc.tile_pool(name="scores", bufs=1, space="PSUM")
        )
        scores_ps = [
            scores_pool.tile([P, C], F32, tag=f"scores{o}", name=f"scores{o}")
            for o in range(CC)
        ]

        rep_ctx = tc.For_i(0, reps, 1) if reps > 1 else None
        if rep_ctx is not None:
            ctx.enter_context(rep_ctx)

        # ---- pass 1: x^T staging, q/k projection, channel scores ------
        # Staging chain (x DMA -> DVE bf16 convert -> XBAR transpose) has
        # ~7us of latency through semaphore hops, so x is prefetched 6
        # tiles ahead and the transpose runs 4 ahead; scores lag q/k by 2
        # tiles so the DVE-add -> Act-relu chain is off the PE critical
        # path.
        with (
            tc.tile_pool(name="proj_ps", bufs=2, space="PSUM") as proj_ps,
            tc.tile_pool(name="xin", bufs=8) as xin,
            tc.tile_pool(name="qk", bufs=4) as qk,
        ):
            def load_x(t):
                x_t = xin.tile([P, C], F32, tag="x", name=f"x_{t}")
                nc.sync.dma_start(x_t[:], x[ts(t, P), :])
                return x_t

            def proj_qk(t):
                qk_ps = proj_ps.tile([P, 2 * C], F32, tag="proj", name=f"qk_{t}")
                if pair_qk:
                    # consecutive matmuls share the xt stationary
                    for o in range(CC):
                        nc.tensor.matmul(
                            qk_ps[:, 0:C], xt_all[:, o, ts(t, P)],
                            wqkv_r[:, o, 0:C],
                            start=(o == 0), stop=(o == CC - 1),
                        )
                        nc.tensor.matmul(
                            qk_ps[:, C : 2 * C], xt_all[:, o, ts(t, P)],
                            wqkv_r[:, o, C : 2 * C],
                            start=(o == 0), stop=(o == CC - 1),
                        )
                else:
                    for o in range(CC):
                        nc.tensor.matmul(
                            qk_ps[:, 0:C], xt_all[:, o, ts(t, P)], wqkv_r[:, o, 0:C],
                            start=(o == 0), stop=(o == CC - 1),
                        )
                    for o in range(CC):
                        nc.tensor.matmul(
                            qk_ps[:, C : 2 * C], xt_all[:, o, ts(t, P)],
                            wqkv_r[:, o, C : 2 * C],
                            start=(o == 0), stop=(o == CC - 1),
                        )
                nc.vector.tensor_tensor(qk_ps[:], qk_ps[:], bias_qk[:], ADD)
                qk_sb = qk.tile([P, 2 * C], BF16, tag="qk", name=f"qks_{t}")
                nc.scalar.activation(qk_sb[:], qk_ps[:], RELU)
                return qk_sb

            def scores_mm(t, qk_sb):
                if probe == "noscores" and 0 < t < NT - 1:
                    return
                for o in range(CC):
                    nc.tensor.matmul(
                        scores_ps[o][:], qk_sb[:, ts(o, P)], qk_sb[:, C : 2 * C],
                        start=(t == 0), stop=(t == NT - 1),
                    )

            if probe == "noxbar":
                hist = {}
                for t in range(NT):
                    hist[t] = proj_qk(t)
                    if t >= 2:
                        scores_mm(t - 2, hist.pop(t - 2))
                scores_mm(NT - 2, hist.pop(NT - 2))
                scores_mm(NT - 1, hist.pop(NT - 1))
            else:
                # prologue (x 0-5 loaded, xt 0-3 transposed) ran before the
                # rep loop; the body stages tiles 4..31 each iteration
                x_tiles = {4: xpr[4], 5: xpr[5]}
                hist = {}
                for t in range(NT):
                    if t + 6 < NT:
                        x_tiles[t + 6] = load_x(t + 6)
                    if t + 4 < NT:
                        prep_xt(t + 4, x_tiles.pop(t + 4))
                    hist[t] = proj_qk(t)
                    if t >= 2:
                        scores_mm(t - 2, hist.pop(t - 2))
                scores_mm(NT - 2, hist.pop(NT - 2))
                scores_mm(NT - 1, hist.pop(NT - 1))

        # ---- softmax + adjacency gate (overlaps pass-2 v matmuls) ------
        # out = (v @ attn) @ Wo is reassociated as v @ (attn @ Wo): the
        # [C,C]x[C,C] product aw costs 16 matmuls vs 128 for y @ Wo,
        # dropping pass-2 PE work from 384 to 272 matmul instructions.
        with (
            tc.tile_pool(name="smx", bufs=8) as smx,
            tc.tile_pool(name="v_ps", bufs=2, space="PSUM") as v_ps_pool,
            tc.tile_pool(name="yo_ps", bufs=2, space="PSUM") as yo_ps_pool,
            tc.tile_pool(name="vt", bufs=5) as vtp,
            tc.tile_pool(name="outp", bufs=3) as outp,
        ):
            def softmax_chunk(o):
                smax = smx.tile([P, 1], F32, tag="smax")
                nc.vector.reduce_max(
                    smax[:], scores_ps[o][:], axis=mybir.AxisListType.X
                )
                nbias = smx.tile([P, 1], F32, tag="nbias")
                nc.vector.tensor_scalar_mul(nbias[:], smax[:], -s)
                ssum = smx.tile([P, 1], F32, tag="ssum")
                attn_e = smx.tile([P, C], F32, tag="attn_e")
                nc.scalar.activation(
                    attn_e[:], scores_ps[o][:],
                    mybir.ActivationFunctionType.Exp,
                    bias=nbias[:], scale=s, accum_out=ssum[:],
                )
                rsum = smx.tile([P, 1], F32, tag="rsum")
                nc.vector.reciprocal(rsum[:], ssum[:])
                attn_r = smx.tile([P, C], F32, tag="attn_r")
                nc.vector.tensor_scalar_mul(attn_r[:], attn_e[:], rsum[:])
                nc.vector.tensor_mul(attn_sb[:, o, :], attn_r[:], adj_sb[:, o, :])
                nc.sync.dma_start_transpose(attn_T[:, :, ts(o, P)], attn_sb[:, o, :])

            def emit_aw():
                for co in range(CC):
                    a_ps = yo_ps_pool.tile([P, C], F32, tag="yo", name=f"aw_{co}")
                    for j in range(CC):
                        nc.tensor.matmul(
                            a_ps[:], attn_T[:, j, ts(co, P)], wo_r[:, j, :],
                            start=(j == 0), stop=(j == CC - 1),
                        )
                    nc.scalar.copy(aw_sb[:, co, :], a_ps[:])

            # ---- pass 2: v^T projection, y = v @ attn, out = y @ Wo + bo
            def emit_v(sl):
                vt_slab = vtp.tile([P, CC, SLAB], BF16, tag="vT", name=f"vt_{sl}")
                for d in range(CC):
                    v_ps = v_ps_pool.tile([P, SLAB], F32, tag="v", name=f"v_{sl}_{d}")
                    for o in range(CC):
                        nc.tensor.matmul(
                            v_ps[:],
                            wqkv_r[:, o, ds(2 * C + d * P, P)],
                            xt_all[:, o, ts(sl, SLAB)],
                            start=(o == 0), stop=(o == CC - 1),
                        )
                    nc.scalar.activation(
                        vt_slab[:, d, :], v_ps[:], RELU, bias=bv[:, d : d + 1]
                    )
                return vt_slab

            def emit_out(sl, vt_slab):
                for tt in range(TPS):
                    t = sl * TPS + tt
                    o_ps = yo_ps_pool.tile([P, C], F32, tag="yo", name=f"o_{sl}_{tt}")
                    for o in range(CC):
                        nc.tensor.matmul(
                            o_ps[:],
                            vt_slab[:, o, ts(tt, P)],
                            aw_sb[:, o, :],
                            start=(o == 0), stop=(o == CC - 1),
                        )
                    out_sb = outp.tile([P, C], F32, tag="out", name=f"os_{sl}_{tt}")
                    if pool_bias:
                        nc.gpsimd.scalar_tensor_tensor(
                            out_sb[:], o_ps[:], 1.0, bo_bc[:], MULT, ADD
                        )
                    else:
                        nc.vector.tensor_tensor(out_sb[:], o_ps[:], bo_bc[:], ADD)
                    st_eng = nc.scalar if act_store else nc.sync
                    st_eng.dma_start(out[ts(t, P), :], out_sb[:])

            vt = {0: emit_v(0)}
            for o in range(CC):
                softmax_chunk(o)
            vt[1] = emit_v(1)
            vt[2] = emit_v(2)
            emit_aw()
            vt[3] = emit_v(3)
            for sl in range(NS):
                emit_out(sl, vt.pop(sl))
                if sl + 4 < NS:
                    vt[sl + 4] = emit_v(sl + 4)

    nc.compile()
    return nc


def _get_nc(reps: int = 1, **kw):
    key = ("nc", reps, tuple(sorted(kw.items())))
    if key not in _CACHE:
        _CACHE[key] = build(reps, **kw)
    return _CACHE[key]


def probe_time(inputs, probe, reps_hi=4096):
    """Timing probe: wall(reps_hi) - wall(1) per extra rep."""
    import time as _t
    walls = {}
    for reps in (1, reps_hi):
        kw = dict(reps=reps)
        if probe:
            kw["probe"] = probe
        w = []
        for _ in range(3):
            t0 = _t.time()
            _run(inputs, **kw)
            w.append(_t.time() - t0)
        walls[reps] = min(w)
    return (walls[reps_hi] - walls[1]) / (reps_hi - 1) * 1e9


def _run(inputs, trace=False, reps: int = 1, **kw):
    nc = _get_nc(reps, **kw)
    x = np.ascontiguousarray(np.asarray(inputs["x"], dtype=np.float32))
    adj = np.ascontiguousarray(np.asarray(inputs["adj"], dtype=np.float32))
    wqkv = np.ascontiguousarray(np.asarray(inputs["Wqkv"], dtype=np.float32))
    bqkv = np.ascontiguousarray(np.asarray(inputs["bqkv"], dtype=np.float32))
    wo = np.ascontiguousarray(np.asarray(inputs["Wo"], dtype=np.float32))
    bo = np.ascontiguousarray(np.asarray(inputs["bo"], dtype=np.float32))

    in_maps = [
        {
            "x": x[b],
            "adj": adj[b],
            "Wqkv": wqkv,
            "bqkv": bqkv,
            "Wo": wo,
            "bo": bo,
        }
        for b in range(B)
    ]
    res = run_bass_kernel_spmd(
        nc, in_maps, core_ids=list(range(B)), trace=trace
    )
    outp = np.stack([res.results[b]["out"] for b in range(B)], axis=0)
    return outp.astype(np.float32), res


def kernel(**inputs) -> np.ndarray:
    out, _ = _run(inputs, trace=False)
    return out


# revision 34
# speedup vs baseline: 1.4908x; 1.4908x over previous
"""ChannelAttention Trainium2 Bass kernel.

Full (unsharded) inputs -> full output. Data-parallel over batch B=8 across
the 8 NeuronCores (one batch element per core, SPMD program, no collectives).

Per-core math (N=4096 tokens, C=512 channels):
    qkv = x @ Wqkv + bqkv ; q,k,v = relu(split(qkv))
    scores = (q^T k) / sqrt(C)           # [C, C] contraction over tokens
    attn = softmax(scores, -1) * adj
    y = v @ attn ; out = y @ Wo + bo

v2 design (PE-bound, ~768 N=512 matmuls):
  - all matmuls in bf16 (PSUM accumulate f32; rel err ~2e-3 vs 2e-2 gate)
  - x^T built with the DMA crossbar transpose (dma_start_transpose, bf16)
    instead of PE transposes: frees ~30us of PE and ~27us of DVE
  - q/k bias folded via DVE add into PSUM + Act relu (kills 64 bias matmuls)
  - scores matmuls run one token-tile behind q/k to hide Act/sem latency
  - v-projection deferred into pass 2 (overlaps the softmax stall) and
    interleaved two slabs ahead of y/out to hide relu/copy latency
"""

import sys

sys.path.insert(0, "/opt/trn_rl_repo")

from contextlib import ExitStack

import numpy as np

import concourse.bass as bass
import concourse.mybir as mybir
import concourse.tile as tile
from concourse import bacc
from concourse.bass import ds, ts
from concourse.bass_utils import run_bass_kernel_spmd

# Problem shape (hardcoded per contract).
B, N, C = 8, 4096, 512
P = 128
CC = C // P            # channel chunks (4)
NT = N // P            # token tiles (32)
NS = 8                 # pass-2 slabs
TPS = NT // NS         # token tiles per slab (4)
SLAB = TPS * P         # tokens per slab (512)

F32 = mybir.dt.float32
BF16 = mybir.dt.bfloat16
ADD = mybir.AluOpType.add
MULT = mybir.AluOpType.mult
RELU = mybir.ActivationFunctionType.Relu

_CACHE = {}


def build(reps: int = 1, probe: str | None = None, pair_qk: bool = True,
          xbar_act: bool = False, pool_bias: bool = False,
          act_store: bool = True):
    # pool_bias=True (Pool-engine bias add via gpsimd scalar_tensor_tensor)
    # fails walrus codegen on this toolchain; kept for reference, off.
    # act_store issues the out-store DMAs from the Act queue to offload SP.
    nc = bacc.Bacc("TRN2", target_bir_lowering=False, debug=False, num_devices=8)

    x = nc.dram_tensor("x", [N, C], F32, kind="ExternalInput").ap()
    adj = nc.dram_tensor("adj", [C, C], F32, kind="ExternalInput").ap()
    wqkv = nc.dram_tensor("Wqkv", [C, 3 * C], F32, kind="ExternalInput").ap()
    bqkv = nc.dram_tensor("bqkv", [3 * C], F32, kind="ExternalInput").ap()
    wo = nc.dram_tensor("Wo", [C, C], F32, kind="ExternalInput").ap()
    bo = nc.dram_tensor("bo", [C], F32, kind="ExternalInput").ap()
    out = nc.dram_tensor("out", [N, C], F32, kind="ExternalOutput").ap()

    s = 1.0 / float(np.sqrt(C))

    with tile.TileContext(nc) as tc, ExitStack() as ctx:
        const = ctx.enter_context(tc.tile_pool(name="const", bufs=1))
        xprime = ctx.enter_context(tc.tile_pool(name="xprime", bufs=6))
        xbp = ctx.enter_context(tc.tile_pool(name="xb", bufs=6))

        xt_all = const.tile([P, CC, N], BF16)    # x^T, channel-major (32KB/part)
        attn_sb = const.tile([P, CC, C], BF16)   # gated softmax rows
        attn_T = const.tile([P, CC, C], BF16)    # attn^T (d-major) via XBAR
        aw_sb = const.tile([P, CC, C], BF16)     # attn @ Wo, channel-major

        def prep_xt(t, x_t):
            x_b = xbp.tile([P, C], BF16, tag="xb", name=f"xb_{t}")
            nc.vector.tensor_copy(x_b[:], x_t[:])
            eng = nc.scalar if xbar_act else nc.sync
            eng.dma_start_transpose(xt_all[:, :, ts(t, P)], x_b[:])

        # ---- constants; emission order = queue order: small consts and
        # the x-pipeline prologue go first so PE/SP unblock early, big
        # weight DMAs after ------------------------------------------------
        with tc.tile_pool(name="stage", bufs=1) as stage:
            brow_f = stage.tile([1, 2 * C], F32, tag="stage_b")
            nc.sync.dma_start(brow_f[:], bqkv[None, 0 : 2 * C])
            brow_r = stage.tile([1, 2 * C], BF16, tag="stage_br")
            nc.vector.tensor_copy(brow_r[:], brow_f[:])

            borow_f = stage.tile([1, C], F32, tag="stage_bo")
            nc.sync.dma_start(borow_f[:], bo[None, :])
            borow_r = stage.tile([1, C], BF16, tag="stage_bor")
            nc.vector.tensor_copy(borow_r[:], borow_f[:])

            ones_f = stage.tile([1, P], F32, tag="stage_ones")
            nc.gpsimd.memset(ones_f[:], 1.0)
            ones_r = stage.tile([1, P], BF16, tag="stage_onesr")
            nc.vector.tensor_copy(ones_r[:], ones_f[:])

            # broadcast biases to [P, *] once (read along free dim later)
            bias_qk = const.tile([P, 2 * C], F32)
            bo_bc = const.tile([P, C], F32)
            with tc.tile_pool(name="bc_ps", bufs=1, space="PSUM") as bc_pool:
                bq_ps = bc_pool.tile([P, C], F32, name="bq_ps", tag="b0")
                nc.tensor.matmul(bq_ps[:], ones_r[:], brow_r[:, 0:C], start=True, stop=True)
                nc.vector.tensor_copy(bias_qk[:, 0:C], bq_ps[:])
                bk_ps = bc_pool.tile([P, C], F32, name="bk_ps", tag="b1")
                nc.tensor.matmul(bk_ps[:], ones_r[:], brow_r[:, C : 2 * C], start=True, stop=True)
                nc.vector.tensor_copy(bias_qk[:, C : 2 * C], bk_ps[:])
                bo_ps = bc_pool.tile([P, C], F32, name="bo_ps", tag="b2")
                nc.tensor.matmul(bo_ps[:], ones_r[:], borow_r[:], start=True, stop=True)
                nc.vector.tensor_copy(bo_bc[:], bo_ps[:])

            # x-pipeline prologue: tiles 0-5 loaded and 0-3 transposed once,
            # outside the rep loop (software-pipeline prologue; x is
            # loop-invariant so iterations 2+ reuse these slices)
            xpr = {}
            for t in range(6):
                xp = xprime.tile([P, C], F32, tag="xp", name=f"xp_{t}")
                nc.sync.dma_start(xp[:], x[ts(t, P), :])
                xpr[t] = xp
            for t in range(4):
                prep_xt(t, xpr[t])

            # big weights after the prologue, issued from the Act queue so
            # they stream in parallel with the x-prologue DMAs on SP
            wqkv_f = stage.tile([P, CC, 3 * C], F32, tag="stage_wqkv")
            wqkv_r = const.tile([P, CC, 3 * C], BF16)
            for o in range(CC):
                nc.scalar.dma_start(
                    wqkv_f[:, o, :], wqkv.rearrange("(o p) d -> p o d", p=P)[:, o, :]
                )
                nc.vector.tensor_copy(wqkv_r[:, o, :], wqkv_f[:, o, :])

            wo_f = stage.tile([P, CC, C], F32, tag="stage_wo")
            nc.scalar.dma_start(wo_f[:], wo.rearrange("(o p) d -> p o d", p=P))
            wo_r = const.tile([P, CC, C], BF16)
            nc.vector.tensor_copy(wo_r[:], wo_f[:])

        # v-bias, per-partition layout [p, chunk]
        bv = const.tile([P, CC], F32)
        nc.sync.dma_start(bv[:], bqkv[2 * C :].rearrange("(o p) -> p o", p=P))

        adj_sb = const.tile([P, CC, C], F32)
        nc.sync.dma_start(adj_sb[:], adj.rearrange("(o p) d -> p o d", p=P))

        if probe == "noxbar":
            # stage x^T once, outside the rep loop (timing probe: removes
            # per-iteration x DMA + convert + XBAR; outputs stay correct)
            with tc.tile_pool(name="xstage", bufs=3) as xst:
                for t in range(NT):
                    xs = xst.tile([P, C], F32, tag="xs", name=f"xs_{t}")
                    nc.sync.dma_start(xs[:], x[ts(t, P), :])
                    xb = xst.tile([P, C], BF16, tag="xsb", name=f"xsb_{t}")
                    nc.vector.tensor_copy(xb[:], xs[:])
                    nc.sync.dma_start_transpose(xt_all[:, :, ts(t, P)], xb[:])

        # ---- per-iteration body ---------------------------------------
        scores_pool = ctx.enter_context(
            tc.tile_pool(name="scores", bufs=1, space="PSUM")
        )
        scores_ps = [
            scores_pool.tile([P, C], F32, tag=f"scores{o}", name=f"scores{o}")
            for o in range(CC)
        ]

        rep_ctx = tc.For_i(0, reps, 1) if reps > 1 else None
        if rep_ctx is not None:
            ctx.enter_context(rep_ctx)

        # ---- pass 1: x^T staging, q/k projection, channel scores ------
        # Staging chain (x DMA -> DVE bf16 convert -> XBAR transpose) has
        # ~7us of latency through semaphore hops, so x is prefetched 6
        # tiles ahead and the transpose runs 4 ahead; scores lag q/k by 2
        # tiles so the DVE-add -> Act-relu chain is off the PE critical
        # path.
        with (
            tc.tile_pool(name="proj_ps", bufs=2, space="PSUM") as proj_ps,
            tc.tile_pool(name="xin", bufs=8) as xin,
            tc.tile_pool(name="qk", bufs=4) as qk,
        ):
            def load_x(t):
                x_t = xin.tile([P, C], F32, tag="x", name=f"x_{t}")
                nc.sync.dma_start(x_t[:], x[ts(t, P), :])
                return x_t

            def proj_qk(t):
                qk_ps = proj_ps.tile([P, 2 * C], F32, tag="proj", name=f"qk_{t}")
                if pair_qk:
                    # consecutive matmuls share the xt stationary
                    for o in range(CC):
                        nc.tensor.matmul(
                            qk_ps[:, 0:C], xt_all[:, o, ts(t, P)],
                            wqkv_r[:, o, 0:C],
                            start=(o == 0), stop=(o == CC - 1),
                        )
                        nc.tensor.matmul(
                            qk_ps[:, C : 2 * C], xt_all[:, o, ts(t, P)],
                            wqkv_r[:, o, C : 2 * C],
                            start=(o == 0), stop=(o == CC - 1),
                        )
                else:
                    for o in range(CC):
                        nc.tensor.matmul(
                            qk_ps[:, 0:C], xt_all[:, o, ts(t, P)], wqkv_r[:, o, 0:C],
                            start=(o == 0), stop=(o == CC - 1),
                        )
                    for o in range(CC):
                        nc.tensor.matmul(
                            qk_ps[:, C : 2 * C], xt_all[:, o, ts(t, P)],
                            wqkv_r[:, o, C : 2 * C],
                            start=(o == 0), stop=(o == CC - 1),
                        )
                nc.vector.tensor_tensor(qk_ps[:], qk_ps[:], bias_qk[:], ADD)
                qk_sb = qk.tile([P, 2 * C], BF16, tag="qk", name=f"qks_{t}")
                nc.scalar.activation(qk_sb[:], qk_ps[:], RELU)
                return qk_sb

            def scores_mm(t, qk_sb):
                if probe == "noscores" and 0 < t < NT - 1:
                    return
                for o in range(CC):
                    nc.tensor.matmul(
                        scores_ps[o][:], qk_sb[:, ts(o, P)], qk_sb[:, C : 2 * C],
                        start=(t == 0), stop=(t == NT - 1),
                    )

            if probe == "noxbar":
                hist = {}
                for t in range(NT):
                    hist[t] = proj_qk(t)
                    if t >= 2:
                        scores_mm(t - 2, hist.pop(t - 2))
                scores_mm(NT - 2, hist.pop(NT - 2))
                scores_mm(NT - 1, hist.pop(NT - 1))
            else:
                # prologue (x 0-5 loaded, xt 0-3 transposed) ran before the
                # rep loop; the body stages tiles 4..31 each iteration
                x_tiles = {4: xpr[4], 5: xpr[5]}
                hist = {}
                for t in range(NT):
                    if t + 6 < NT:
                        x_tiles[t + 6] = load_x(t + 6)
                    if t + 4 < NT:
                        prep_xt(t + 4, x_tiles.pop(t + 4))
                    hist[t] = proj_qk(t)
                    if t >= 2:
                        scores_mm(t - 2, hist.pop(t - 2))
                scores_mm(NT - 2, hist.pop(NT - 2))
                scores_mm(NT - 1, hist.pop(NT - 1))

        # ---- softmax + adjacency gate (overlaps pass-2 v matmuls) ------
        # out = (v @ attn) @ Wo is reassociated as v @ (attn @ Wo): the
        # [C,C]x[C,C] product aw costs 16 matmuls vs 128 for y @ Wo,
        # dropping pass-2 PE work from 384 to 272 matmul instructions.
        with (
            tc.tile_pool(name="smx", bufs=8) as smx,
            tc.tile_pool(name="v_ps", bufs=2, space="PSUM") as v_ps_pool,
            tc.tile_pool(name="yo_ps", bufs=2, space="PSUM") as yo_ps_pool,
            tc.tile_pool(name="vt", bufs=5) as vtp,
            tc.tile_pool(name="outp", bufs=3) as outp,
        ):
            def softmax_chunk(o):
                smax = smx.tile([P, 1], F32, tag="smax")
                nc.vector.reduce_max(
                    smax[:], scores_ps[o][:], axis=mybir.AxisListType.X
                )
                nbias = smx.tile([P, 1], F32, tag="nbias")
                nc.vector.tensor_scalar_mul(nbias[:], smax[:], -s)
                ssum = smx.tile([P, 1], F32, tag="ssum")
                attn_e = smx.tile([P, C], F32, tag="attn_e")
                nc.scalar.activation(
                    attn_e[:], scores_ps[o][:],
                    mybir.ActivationFunctionType.Exp,
                    bias=nbias[:], scale=s, accum_out=ssum[:],
                )
                rsum = smx.tile([P, 1], F32, tag="rsum")
                nc.vector.reciprocal(rsum[:], ssum[:])
                attn_r = smx.tile([P, C], F32, tag="attn_r")
                nc.vector.tensor_scalar_mul(attn_r[:], attn_e[:], rsum[:])
                nc.vector.tensor_mul(attn_sb[:, o, :], attn_r[:], adj_sb[:, o, :])
                nc.sync.dma_start_transpose(attn_T[:, :, ts(o, P)], attn_sb[:, o, :])

            def emit_aw():
                for co in range(CC):
                    a_ps = yo_ps_pool.tile([P, C], F32, tag="yo", name=f"aw_{co}")
                    for j in range(CC):
                        nc.tensor.matmul(
                            a_ps[:], attn_T[:, j, ts(co, P)], wo_r[:, j, :],
                            start=(j == 0), stop=(j == CC - 1),
                        )
                    nc.scalar.copy(aw_sb[:, co, :], a_ps[:])

            # ---- pass 2: v^T projection, y = v @ attn, out = y @ Wo + bo
            def emit_v(sl):
                vt_slab = vtp.tile([P, CC, SLAB], BF16, tag="vT", name=f"vt_{sl}")
                for d in range(CC):
                    v_ps = v_ps_pool.tile([P, SLAB], F32, tag="v", name=f"v_{sl}_{d}")
                    for o in range(CC):
                        nc.tensor.matmul(
                            v_ps[:],
                            wqkv_r[:, o, ds(2 * C + d * P, P)],
                            xt_all[:, o, ts(sl, SLAB)],
                            start=(o == 0), stop=(o == CC - 1),
                        )
                    nc.scalar.activation(
                        vt_slab[:, d, :], v_ps[:], RELU, bias=bv[:, d : d + 1]
                    )
                return vt_slab

            def emit_out(sl, vt_slab):
                for tt in range(TPS):
                    t = sl * TPS + tt
                    o_ps = yo_ps_pool.tile([P, C], F32, tag="yo", name=f"o_{sl}_{tt}")
                    for o in range(CC):
                        nc.tensor.matmul(
                            o_ps[:],
                            vt_slab[:, o, ts(tt, P)],
                            aw_sb[:, o, :],
                            start=(o == 0), stop=(o == CC - 1),
                        )
                    out_sb = outp.tile([P, C], F32, tag="out", name=f"os_{sl}_{tt}")
                    if pool_bias:
                        nc.gpsimd.scalar_tensor_tensor(
                            out_sb[:], o_ps[:], 1.0, bo_bc[:], MULT, ADD
                        )
                    else:
                        nc.vector.tensor_tensor(out_sb[:], o_ps[:], bo_bc[:], ADD)
                    st_eng = nc.scalar if act_store else nc.sync
                    st_eng.dma_start(out[ts(t, P), :], out_sb[:])

            vt = {0: emit_v(0)}
            for o in range(CC):
                softmax_chunk(o)
            vt[1] = emit_v(1)
            vt[2] = emit_v(2)
            emit_aw()
            vt[3] = emit_v(3)
            for sl in range(NS):
                emit_out(sl, vt.pop(sl))
                if sl + 4 < NS:
                    vt[sl + 4] = emit_v(sl + 4)

    nc.compile()
    return nc


def _get_nc(reps: int = 1, **kw):
    key = ("nc", reps, tuple(sorted(kw.items())))
    if key not in _CACHE:
        _CACHE[key] = build(reps, **kw)
    return _CACHE[key]


def probe_time(inputs, probe, reps_hi=4096):
    """Timing probe: wall(reps_hi) - wall(1) per extra rep."""
    import time as _t
    walls = {}
    for reps in (1, reps_hi):
        kw = dict(reps=reps)
        if probe:
            kw["probe"] = probe
        w = []
        for _ in range(3):
            t0 = _t.time()
            _run(inputs, **kw)
            w.append(_t.time() - t0)
        walls[reps] = min(w)
    return (walls[reps_hi] - walls[1]) / (reps_hi - 1) * 1e9


def _run(inputs, trace=False, reps: int = 1, **kw):
    nc = _get_nc(reps, **kw)
    x = np.ascontiguousarray(np.asarray(inputs["x"], dtype=np.float32))
    adj = np.ascontiguousarray(np.asarray(inputs["adj"], dtype=np.float32))
    wqkv = np.ascontiguousarray(np.asarray(inputs["Wqkv"], dtype=np.float32))
    bqkv = np.ascontiguousarray(np.asarray(inputs["bqkv"], dtype=np.float32))
    wo = np.ascontiguousarray(np.asarray(inputs["Wo"], dtype=np.float32))
    bo = np.ascontiguousarray(np.asarray(inputs["bo"], dtype=np.float32))

    in_maps = [
        {
            "x": x[b],
            "adj": adj[b],
            "Wqkv": wqkv,
            "bqkv": bqkv,
            "Wo": wo,
            "bo": bo,
        }
        for b in range(B)
    ]
    res = run_bass_kernel_spmd(
        nc, in_maps, core_ids=list(range(B)), trace=trace
    )
    outp = np.stack([res.results[b]["out"] for b in range(B)], axis=0)
    return outp.astype(np.float32), res


def kernel(**inputs) -> np.ndarray:
    out, _ = _run(inputs, trace=False)
    return out


# revision 35
# speedup vs baseline: 2.1006x; 1.4091x over previous
"""ChannelAttention Trainium2 Bass kernel.

Full (unsharded) inputs -> full output. Data-parallel over batch B=8 across
the 8 NeuronCores (one batch element per core, SPMD program, no collectives).

Per-core math (N=4096 tokens, C=512 channels):
    qkv = x @ Wqkv + bqkv ; q,k,v = relu(split(qkv))
    scores = (q^T k) / sqrt(C)           # [C, C] contraction over tokens
    attn = softmax(scores, -1) * adj
    y = v @ attn ; out = y @ Wo + bo

v2 design (PE-bound, ~768 N=512 matmuls):
  - all matmuls in bf16 (PSUM accumulate f32; rel err ~2e-3 vs 2e-2 gate)
  - x^T built with the DMA crossbar transpose (dma_start_transpose, bf16)
    instead of PE transposes: frees ~30us of PE and ~27us of DVE
  - q/k bias folded via DVE add into PSUM + Act relu (kills 64 bias matmuls)
  - scores matmuls run one token-tile behind q/k to hide Act/sem latency
  - v-projection deferred into pass 2 (overlaps the softmax stall) and
    interleaved two slabs ahead of y/out to hide relu/copy latency
"""

import sys

sys.path.insert(0, "/opt/trn_rl_repo")

from contextlib import ExitStack

import numpy as np

import concourse.bass as bass
import concourse.mybir as mybir
import concourse.tile as tile
from concourse import bacc
from concourse.bass import ds, ts
from concourse.bass_utils import run_bass_kernel_spmd

# Problem shape (hardcoded per contract).
B, N, C = 8, 4096, 512
P = 128
CC = C // P            # channel chunks (4)
NT = N // P            # token tiles (32)
NS = 8                 # pass-2 slabs
TPS = NT // NS         # token tiles per slab (4)
SLAB = TPS * P         # tokens per slab (512)

F32 = mybir.dt.float32
BF16 = mybir.dt.bfloat16
ADD = mybir.AluOpType.add
MULT = mybir.AluOpType.mult
RELU = mybir.ActivationFunctionType.Relu

_CACHE = {}


def build(reps: int = 1, probe: str | None = None, pair_qk: bool = True,
          xbar_act: bool = False, pool_bias: bool = False,
          act_store: bool = True):
    # pool_bias=True (Pool-engine bias add via gpsimd scalar_tensor_tensor)
    # fails walrus codegen on this toolchain; kept for reference, off.
    # act_store issues the out-store DMAs from the Act queue to offload SP.
    nc = bacc.Bacc("TRN2", target_bir_lowering=False, debug=False, num_devices=8)

    x = nc.dram_tensor("x", [N, C], F32, kind="ExternalInput").ap()
    adj = nc.dram_tensor("adj", [C, C], F32, kind="ExternalInput").ap()
    wqkv = nc.dram_tensor("Wqkv", [C, 3 * C], F32, kind="ExternalInput").ap()
    bqkv = nc.dram_tensor("bqkv", [3 * C], F32, kind="ExternalInput").ap()
    wo = nc.dram_tensor("Wo", [C, C], F32, kind="ExternalInput").ap()
    bo = nc.dram_tensor("bo", [C], F32, kind="ExternalInput").ap()
    out = nc.dram_tensor("out", [N, C], F32, kind="ExternalOutput").ap()

    s = 1.0 / float(np.sqrt(C))

    with tile.TileContext(nc) as tc, ExitStack() as ctx:
        const = ctx.enter_context(tc.tile_pool(name="const", bufs=1))
        xprime = ctx.enter_context(tc.tile_pool(name="xprime", bufs=6))
        xbp = ctx.enter_context(tc.tile_pool(name="xb", bufs=8))

        xt_all = const.tile([P, CC, N], BF16)    # x^T, channel-major (32KB/part)
        attn_sb = const.tile([P, CC, C], BF16)   # gated softmax rows
        attn_T = const.tile([P, CC, C], BF16)    # attn^T (d-major) via XBAR
        aw_sb = const.tile([P, CC, C], BF16)     # attn @ Wo, channel-major

        def prep_xt(t, x_t):
            x_b = xbp.tile([P, C], BF16, tag="xb", name=f"xb_{t}")
            nc.vector.tensor_copy(x_b[:], x_t[:])
            eng = nc.scalar if xbar_act else nc.sync
            eng.dma_start_transpose(xt_all[:, :, ts(t, P)], x_b[:])

        # ---- constants; emission order = queue order: small consts and
        # the x-pipeline prologue go first so PE/SP unblock early, big
        # weight DMAs after ------------------------------------------------
        with tc.tile_pool(name="stage", bufs=1) as stage:
            brow_f = stage.tile([1, 2 * C], F32, tag="stage_b")
            nc.sync.dma_start(brow_f[:], bqkv[None, 0 : 2 * C])
            brow_r = stage.tile([1, 2 * C], BF16, tag="stage_br")
            nc.vector.tensor_copy(brow_r[:], brow_f[:])

            borow_f = stage.tile([1, C], F32, tag="stage_bo")
            nc.sync.dma_start(borow_f[:], bo[None, :])
            borow_r = stage.tile([1, C], BF16, tag="stage_bor")
            nc.vector.tensor_copy(borow_r[:], borow_f[:])

            ones_f = stage.tile([1, P], F32, tag="stage_ones")
            nc.gpsimd.memset(ones_f[:], 1.0)
            ones_r = stage.tile([1, P], BF16, tag="stage_onesr")
            nc.vector.tensor_copy(ones_r[:], ones_f[:])

            # broadcast biases to [P, *] once (read along free dim later)
            bias_qk = const.tile([P, 2 * C], F32)
            bo_bc = const.tile([P, C], F32)
            with tc.tile_pool(name="bc_ps", bufs=1, space="PSUM") as bc_pool:
                bq_ps = bc_pool.tile([P, C], F32, name="bq_ps", tag="b0")
                nc.tensor.matmul(bq_ps[:], ones_r[:], brow_r[:, 0:C], start=True, stop=True)
                nc.vector.tensor_copy(bias_qk[:, 0:C], bq_ps[:])
                bk_ps = bc_pool.tile([P, C], F32, name="bk_ps", tag="b1")
                nc.tensor.matmul(bk_ps[:], ones_r[:], brow_r[:, C : 2 * C], start=True, stop=True)
                nc.vector.tensor_copy(bias_qk[:, C : 2 * C], bk_ps[:])
                bo_ps = bc_pool.tile([P, C], F32, name="bo_ps", tag="b2")
                nc.tensor.matmul(bo_ps[:], ones_r[:], borow_r[:], start=True, stop=True)
                nc.vector.tensor_copy(bo_bc[:], bo_ps[:])

            # x-pipeline prologue: tiles 0-5 loaded and 0-3 transposed once,
            # outside the rep loop (software-pipeline prologue; x is
            # loop-invariant so iterations 2+ reuse these slices)
            xpr = {}
            for t in range(6):
                xp = xprime.tile([P, C], F32, tag="xp", name=f"xp_{t}")
                nc.sync.dma_start(xp[:], x[ts(t, P), :])
                xpr[t] = xp
            for t in range(4):
                prep_xt(t, xpr[t])

            # big weights after the prologue, issued from the Act queue so
            # they stream in parallel with the x-prologue DMAs on SP
            wqkv_f = stage.tile([P, CC, 3 * C], F32, tag="stage_wqkv")
            wqkv_r = const.tile([P, CC, 3 * C], BF16)
            for o in range(CC):
                nc.scalar.dma_start(
                    wqkv_f[:, o, :], wqkv.rearrange("(o p) d -> p o d", p=P)[:, o, :]
                )
                nc.vector.tensor_copy(wqkv_r[:, o, :], wqkv_f[:, o, :])

            wo_f = stage.tile([P, CC, C], F32, tag="stage_wo")
            nc.scalar.dma_start(wo_f[:], wo.rearrange("(o p) d -> p o d", p=P))
            wo_r = const.tile([P, CC, C], BF16)
            nc.vector.tensor_copy(wo_r[:], wo_f[:])

        # v-bias, per-partition layout [p, chunk]
        bv = const.tile([P, CC], F32)
        nc.sync.dma_start(bv[:], bqkv[2 * C :].rearrange("(o p) -> p o", p=P))

        adj_sb = const.tile([P, CC, C], F32)
        nc.sync.dma_start(adj_sb[:], adj.rearrange("(o p) d -> p o d", p=P))

        if probe == "noxbar":
            # stage x^T once, outside the rep loop (timing probe: removes
            # per-iteration x DMA + convert + XBAR; outputs stay correct)
            with tc.tile_pool(name="xstage", bufs=3) as xst:
                for t in range(NT):
                    xs = xst.tile([P, C], F32, tag="xs", name=f"xs_{t}")
                    nc.sync.dma_start(xs[:], x[ts(t, P), :])
                    xb = xst.tile([P, C], BF16, tag="xsb", name=f"xsb_{t}")
                    nc.vector.tensor_copy(xb[:], xs[:])
                    nc.sync.dma_start_transpose(xt_all[:, :, ts(t, P)], xb[:])

        # ---- per-iteration body ---------------------------------------
        scores_pool = ctx.enter_context(
            tc.tile_pool(name="scores", bufs=1, space="PSUM")
        )
        scores_ps = [
            scores_pool.tile([P, C], F32, tag=f"scores{o}", name=f"scores{o}")
            for o in range(CC)
        ]

        rep_ctx = tc.For_i(0, reps, 1) if reps > 1 else None
        if rep_ctx is not None:
            ctx.enter_context(rep_ctx)

        # ---- pass 1: x^T staging, q/k projection, channel scores ------
        # Staging chain (x DMA -> DVE bf16 convert -> XBAR transpose) has
        # ~7us of latency through semaphore hops, so x is prefetched 6
        # tiles ahead and the transpose runs 4 ahead; scores lag q/k by 2
        # tiles so the DVE-add -> Act-relu chain is off the PE critical
        # path.
        with (
            tc.tile_pool(name="proj_ps", bufs=2, space="PSUM") as proj_ps,
            tc.tile_pool(name="xin", bufs=10) as xin,
            tc.tile_pool(name="qk", bufs=6) as qk,
        ):
            def load_x(t):
                x_t = xin.tile([P, C], F32, tag="x", name=f"x_{t}")
                nc.sync.dma_start(x_t[:], x[ts(t, P), :])
                return x_t

            def proj_qk(t):
                qk_ps = proj_ps.tile([P, 2 * C], F32, tag="proj", name=f"qk_{t}")
                if pair_qk:
                    # consecutive matmuls share the xt stationary
                    for o in range(CC):
                        nc.tensor.matmul(
                            qk_ps[:, 0:C], xt_all[:, o, ts(t, P)],
                            wqkv_r[:, o, 0:C],
                            start=(o == 0), stop=(o == CC - 1),
                        )
                        nc.tensor.matmul(
                            qk_ps[:, C : 2 * C], xt_all[:, o, ts(t, P)],
                            wqkv_r[:, o, C : 2 * C],
                            start=(o == 0), stop=(o == CC - 1),
                        )
                else:
                    for o in range(CC):
                        nc.tensor.matmul(
                            qk_ps[:, 0:C], xt_all[:, o, ts(t, P)], wqkv_r[:, o, 0:C],
                            start=(o == 0), stop=(o == CC - 1),
                        )
                    for o in range(CC):
                        nc.tensor.matmul(
                            qk_ps[:, C : 2 * C], xt_all[:, o, ts(t, P)],
                            wqkv_r[:, o, C : 2 * C],
                            start=(o == 0), stop=(o == CC - 1),
                        )
                nc.vector.tensor_tensor(qk_ps[:], qk_ps[:], bias_qk[:], ADD)
                qk_sb = qk.tile([P, 2 * C], BF16, tag="qk", name=f"qks_{t}")
                nc.scalar.activation(qk_sb[:], qk_ps[:], RELU)
                return qk_sb

            def scores_mm(t, qk_sb):
                if probe == "noscores" and 0 < t < NT - 1:
                    return
                for o in range(CC):
                    nc.tensor.matmul(
                        scores_ps[o][:], qk_sb[:, ts(o, P)], qk_sb[:, C : 2 * C],
                        start=(t == 0), stop=(t == NT - 1),
                    )

            if probe == "noxbar":
                hist = {}
                for t in range(NT):
                    hist[t] = proj_qk(t)
                    if t >= 2:
                        scores_mm(t - 2, hist.pop(t - 2))
                scores_mm(NT - 2, hist.pop(NT - 2))
                scores_mm(NT - 1, hist.pop(NT - 1))
            else:
                # prologue (x 0-5 loaded, xt 0-3 transposed) ran before the
                # rep loop; the body stages tiles 4..31 each iteration
                x_tiles = {4: xpr[4], 5: xpr[5]}
                hist = {}
                for t in range(NT):
                    if t + 6 < NT:
                        x_tiles[t + 6] = load_x(t + 6)
                    if t + 4 < NT:
                        prep_xt(t + 4, x_tiles.pop(t + 4))
                    hist[t] = proj_qk(t)
                    if t >= 2:
                        scores_mm(t - 2, hist.pop(t - 2))
                scores_mm(NT - 2, hist.pop(NT - 2))
                scores_mm(NT - 1, hist.pop(NT - 1))

        # ---- softmax + adjacency gate (overlaps pass-2 v matmuls) ------
        # out = (v @ attn) @ Wo is reassociated as v @ (attn @ Wo): the
        # [C,C]x[C,C] product aw costs 16 matmuls vs 128 for y @ Wo,
        # dropping pass-2 PE work from 384 to 272 matmul instructions.
        with (
            tc.tile_pool(name="smx", bufs=8) as smx,
            tc.tile_pool(name="v_ps", bufs=2, space="PSUM") as v_ps_pool,
            tc.tile_pool(name="yo_ps", bufs=2, space="PSUM") as yo_ps_pool,
            tc.tile_pool(name="vt", bufs=6) as vtp,
            tc.tile_pool(name="outp", bufs=4) as outp,
        ):
            def softmax_chunk(o):
                smax = smx.tile([P, 1], F32, tag="smax")
                nc.vector.reduce_max(
                    smax[:], scores_ps[o][:], axis=mybir.AxisListType.X
                )
                nbias = smx.tile([P, 1], F32, tag="nbias")
                nc.vector.tensor_scalar_mul(nbias[:], smax[:], -s)
                ssum = smx.tile([P, 1], F32, tag="ssum")
                attn_e = smx.tile([P, C], F32, tag="attn_e")
                nc.scalar.activation(
                    attn_e[:], scores_ps[o][:],
                    mybir.ActivationFunctionType.Exp,
                    bias=nbias[:], scale=s, accum_out=ssum[:],
                )
                rsum = smx.tile([P, 1], F32, tag="rsum")
                nc.vector.reciprocal(rsum[:], ssum[:])
                attn_r = smx.tile([P, C], F32, tag="attn_r")
                nc.vector.tensor_scalar_mul(attn_r[:], attn_e[:], rsum[:])
                nc.vector.tensor_mul(attn_sb[:, o, :], attn_r[:], adj_sb[:, o, :])
                nc.sync.dma_start_transpose(attn_T[:, :, ts(o, P)], attn_sb[:, o, :])

            def emit_aw():
                for co in range(CC):
                    a_ps = yo_ps_pool.tile([P, C], F32, tag="yo", name=f"aw_{co}")
                    for j in range(CC):
                        nc.tensor.matmul(
                            a_ps[:], attn_T[:, j, ts(co, P)], wo_r[:, j, :],
                            start=(j == 0), stop=(j == CC - 1),
                        )
                    nc.scalar.copy(aw_sb[:, co, :], a_ps[:])

            # ---- pass 2: v^T projection, y = v @ attn, out = y @ Wo + bo
            def emit_v(sl):
                vt_slab = vtp.tile([P, CC, SLAB], BF16, tag="vT", name=f"vt_{sl}")
                for d in range(CC):
                    v_ps = v_ps_pool.tile([P, SLAB], F32, tag="v", name=f"v_{sl}_{d}")
                    for o in range(CC):
                        nc.tensor.matmul(
                            v_ps[:],
                            wqkv_r[:, o, ds(2 * C + d * P, P)],
                            xt_all[:, o, ts(sl, SLAB)],
                            start=(o == 0), stop=(o == CC - 1),
                        )
                    nc.scalar.activation(
                        vt_slab[:, d, :], v_ps[:], RELU, bias=bv[:, d : d + 1]
                    )
                return vt_slab

            def emit_out(sl, vt_slab):
                for tt in range(TPS):
                    t = sl * TPS + tt
                    o_ps = yo_ps_pool.tile([P, C], F32, tag="yo", name=f"o_{sl}_{tt}")
                    for o in range(CC):
                        nc.tensor.matmul(
                            o_ps[:],
                            vt_slab[:, o, ts(tt, P)],
                            aw_sb[:, o, :],
                            start=(o == 0), stop=(o == CC - 1),
                        )
                    out_sb = outp.tile([P, C], F32, tag="out", name=f"os_{sl}_{tt}")
                    if pool_bias:
                        nc.gpsimd.scalar_tensor_tensor(
                            out_sb[:], o_ps[:], 1.0, bo_bc[:], MULT, ADD
                        )
                    else:
                        nc.vector.tensor_tensor(out_sb[:], o_ps[:], bo_bc[:], ADD)
                    st_eng = nc.scalar if act_store else nc.sync
                    st_eng.dma_start(out[ts(t, P), :], out_sb[:])

            vt = {0: emit_v(0)}
            for o in range(CC):
                softmax_chunk(o)
            vt[1] = emit_v(1)
            vt[2] = emit_v(2)
            emit_aw()
            vt[3] = emit_v(3)
            for sl in range(NS):
                emit_out(sl, vt.pop(sl))
                if sl + 4 < NS:
                    vt[sl + 4] = emit_v(sl + 4)

    nc.compile()
    return nc


def _get_nc(reps: int = 1, **kw):
    key = ("nc", reps, tuple(sorted(kw.items())))
    if key not in _CACHE:
        _CACHE[key] = build(reps, **kw)
    return _CACHE[key]


def probe_time(inputs, probe, reps_hi=4096):
    """Timing probe: wall(reps_hi) - wall(1) per extra rep."""
    import time as _t
    walls = {}
    for reps in (1, reps_hi):
        kw = dict(reps=reps)
        if probe:
            kw["probe"] = probe
        w = []
        for _ in range(3):
            t0 = _t.time()
            _run(inputs, **kw)
            w.append(_t.time() - t0)
        walls[reps] = min(w)
    return (walls[reps_hi] - walls[1]) / (reps_hi - 1) * 1e9


def _run(inputs, trace=False, reps: int = 1, **kw):
    nc = _get_nc(reps, **kw)
    x = np.ascontiguousarray(np.asarray(inputs["x"], dtype=np.float32))
    adj = np.ascontiguousarray(np.asarray(inputs["adj"], dtype=np.float32))
    wqkv = np.ascontiguousarray(np.asarray(inputs["Wqkv"], dtype=np.float32))
    bqkv = np.ascontiguousarray(np.asarray(inputs["bqkv"], dtype=np.float32))
    wo = np.ascontiguousarray(np.asarray(inputs["Wo"], dtype=np.float32))
    bo = np.ascontiguousarray(np.asarray(inputs["bo"], dtype=np.float32))

    in_maps = [
        {
            "x": x[b],
            "adj": adj[b],
            "Wqkv": wqkv,
            "bqkv": bqkv,
            "Wo": wo,
            "bo": bo,
        }
        for b in range(B)
    ]
    res = run_bass_kernel_spmd(
        nc, in_maps, core_ids=list(range(B)), trace=trace
    )
    outp = np.stack([res.results[b]["out"] for b in range(B)], axis=0)
    return outp.astype(np.float32), res


def kernel(**inputs) -> np.ndarray:
    out, _ = _run(inputs, trace=False)
    return out
